# revision 5
# baseline (speedup 1.0000x reference)
"""Causal multi-head attention (B=4, H=16, S=2048, D=64) on 8 TRN2 NeuronCores.

Sharding: B*H = 64 (batch, head) pairs -> 8 per core, fully independent,
no collectives.

v2 design (from trace analysis of the 174us v1):
  v1 engine busy per core: DVE 125us (2-pass Schraudolph 54 + trimask 38
  + normalize 32), Scalar 114us (exp), PE 118us matmul (+72us LDWEIGHTS,
  fully overlapped per trace). Everything near-saturated; wall = 174us.

  Work cuts in v2:
  - Host pre-casts Q,K,V to bf16 and pre-transposes Q,K to [64, S]
    (no 128-pad): input DMA drops 36MB -> ~10MB per core, and all DMAs
    become cast-free so they run on HWDGE (sync engine) instead of
    gpsimd SWDGE. Pad rows 64:128 of the static Q/K SBUF tiles are
    zeroed once by gpsimd.
  - DVE exp is ONE pass: i16 Schraudolph. bits16 = round(A16*s + B16)
    written via f32->i16 convert into the bf16 ut tile (bitcast I16).
    bf16 bits have the same layout as the f32 top half, so this is the
    classic exp bit-hack at half width (~2% rms, dominated by the
    Schraudolph approx itself, same as v1's 2-pass i32 path).
  - The causal diagonal-block mask is FOLDED INTO the exp: the first
    256 cols of each key-block row go through DVE scalar_tensor_tensor
    (ps * A16) + BMASK, where BMASK holds B16 on the kept triangle and
    B16 - A16*600 on the masked one (masked probs ~1e-31). The v1
    trimask multiply (38us DVE) disappears.
  - Normalize: DVE reciprocal of the matmul-accumulated denominator
    column, multiply on gpsimd (idle otherwise). PSUM O tiles pack two
    q-blocks per bank so 4 q-blocks are in flight.
  - exp work is split scalar/DVE per-slot by a static greedy balance
    (~88us each), under the PE's ~100us.
  - PV (one head behind QK) is interleaved after EVERY 1024-col score
    tile (~6 pairs per slot): with 3 exp engines the PE is the
    bottleneck, so keep its stream dense; PV pairs are always
    dependency-ready (head h-1 fully exp'd).
"""

import numpy as np

import concourse.bass as bass
import concourse.tile as tile
from concourse import mybir
from concourse.bass_utils import run_bass_kernel_spmd
from concourse.vector_clock import ScopedClock, VectorClock

F32 = mybir.dt.float32
BF16 = mybir.dt.bfloat16
I16 = mybir.dt.int16

B, H, S, D = 4, 16, 2048, 64
N_CORES = 8
HEADS_PER_CORE = B * H // N_CORES  # 8
NB = S // 128  # 16 key blocks of 128
SCALE = 1.0 / np.sqrt(np.float32(D))  # 0.125
DIAGW = 256  # width of the fused-mask DVE slot at the head of each kb row

# i16 Schraudolph: bits16 = round(A16*s + B16) viewed as bf16 ~ exp(s/8)
A16 = 0.125 * float(np.log2(np.e)) * 128.0  # 23.0831
B16 = (127.0 - 0.0440) * 128.0  # 16250.368
MASK_BIAS = -600.0  # exp(0.125*(s-600)) ~ 1e-33: dead but positive bf16
MASKB = B16 + A16 * MASK_BIAS  # ~2400.5: tiny positive bf16 bits

# per-slot engine cost model (ns) used for the static scalar/DVE balance
_SC_NS = lambda w: 0.833 * w + 90.0
_DV_NS = lambda w: 1.07 * w + 170.0


def _plan_slots():
    """Per kb: list of (c0, w, engine) exp slots; engine in {'diag','S','V'}.
    Greedy-balance the flexible slots across Scalar and DVE given each
    engine's fixed load (DVE: diag slots + paired reciprocals; Scalar:
    normalize multiplies)."""
    slots = {}
    flex = []
    dve_t = 8 * 120.0  # paired reciprocals
    sc_t = 16 * 145.0  # normalize multiplies (activation Copy w/ scale)
    for kb in range(NB):
        L = S - kb * 128
        dw = min(DIAGW, L)
        slots[kb] = [(0, dw, "diag")]
        dve_t += _DV_NS(dw)
        c = dw
        while c < L:
            # flex chunks end at ps-tile boundaries (multiples of 1024)
            w = min(1024 * (c // 1024 + 1), L) - c
            flex.append((kb, c, w))
            c += w
    for kb, c, w in sorted(flex, key=lambda t: -t[2]):
        if sc_t + _SC_NS(w) <= dve_t + _DV_NS(w):
            slots[kb].append((c, w, "S"))
            sc_t += _SC_NS(w)
        else:
            slots[kb].append((c, w, "V"))
            dve_t += _DV_NS(w)
    for kb in slots:
        slots[kb].sort()
    return slots


SLOT_PLAN = _plan_slots()


def _patch_tile_drain():
    """This walrus build rejects >1 sem wait on the kernel-tail Drain
    instruction ("Too many sync wait commands"). Spread the waits across
    single-wait NOPs on the sync engine instead."""
    if getattr(tile.TileContext, "_drain_patched", False):
        return

    def _drain_and_barrier(self, tick_clock, wait_clock):
        gc = tick_clock.global_clock
        n = len(gc)
        for i in range(n):
            if gc[i] > 0:
                vc = VectorClock([gc[j] if j == i else 0 for j in range(n)])
                nop_inst = self.nc.sync.nop(nofuse=True, hint=f"drainwait{i}")
                wait_clock.add_sem_waits(nop_inst.ins, ScopedClock({None: vc}))
        self.nc.sync.drain()
        self.nc.all_engine_barrier()
        popped = self.nc._tile_sem_poison_stack.pop()
        assert popped is self._sem_poison
        self.nc.clear_and_free_semaphores(list(self.sems.allocated().values()))
        self.nc.all_engine_barrier()

    tile.TileContext._drain_and_barrier = _drain_and_barrier
    tile.TileContext._drain_patched = True


_patch_tile_drain()


def _split_multi_waits(nc, limit=1):
    """This walrus build allows at most one sem wait per instruction.
    Move excess waits onto same-engine NOPs inserted just before."""
    ctr = [0]
    for func in nc.m.functions:
        for bb in func.blocks:
            insts = list(bb.instructions)
            out = []
            changed = False
            for inst in insts:
                si = inst.sync_info
                if si is not None and si.on_wait is not None and len(si.on_wait) > limit:
                    waits = list(si.on_wait)
                    extra, keep = waits[:-limit], waits[-limit:]
                    for w in extra:
                        ctr[0] += 1
                        nop = mybir.InstNoOp(
                            name=f"waitsplit-{ctr[0]}", ins=[], outs=[]
                        )
                        nop.engine = inst.engine
                        nop.sync_info = mybir.SyncInfo(on_wait=[w], on_update=[])
                        out.append(nop)
                    inst.sync_info = mybir.SyncInfo(
                        on_wait=keep, on_update=list(si.on_update or [])
                    )
                    changed = True
                out.append(inst)
            if changed:
                try:
                    bb.instructions[:] = out
                except Exception:
                    bb.instructions = out
    return nc


def build_nc(n_heads: int = HEADS_PER_CORE):
    nc = bass.Bass("TRN2", target_bir_lowering=False)
    qt_d = nc.dram_tensor("queriesT", [n_heads, 64, S], BF16, kind="ExternalInput")
    kt_d = nc.dram_tensor("keysT", [n_heads, 64, S], BF16, kind="ExternalInput")
    v_d = nc.dram_tensor("values", [n_heads, S, D], BF16, kind="ExternalInput")
    o_d = nc.dram_tensor("out", [n_heads, S, D], F32, kind="ExternalOutput")

    # [h, p, n, d] view of v / out: s = n*128 + p
    v_r = v_d[:].rearrange("h (n p) d -> h p n d", p=128)
    o_r = o_d[:].rearrange("h (n p) d -> h p n d", p=128)

    N_VBUF = 4

    with tile.TileContext(nc) as tc:
        with (
            tc.tile_pool(name="const", bufs=1) as constp,
            tc.tile_pool(name="ut", bufs=2) as utp,
            tc.tile_pool(name="rz", bufs=4) as rzp,
            tc.tile_pool(name="ps_s", bufs=3, space="PSUM") as ps_s,
            tc.tile_pool(name="ps_o", bufs=2, space="PSUM") as ps_o,
        ):
            # ---- persistent SBUF tiles -------------------------------
            qts = [
                constp.tile([128, S], BF16, name=f"qt{i}", tag=f"qt{i}")
                for i in range(2)
            ]
            kts = [
                constp.tile([128, S], BF16, name=f"kt{i}", tag=f"kt{i}")
                for i in range(2)
            ]
            vps = [
                constp.tile([128, NB, D + 2], BF16, name=f"vp{i}", tag=f"vp{i}")
                for i in range(N_VBUF)
            ]
            ohs = [
                constp.tile([128, NB, D], F32, name=f"oh{i}", tag=f"oh{i}")
                for i in range(2)
            ]
            bmask = constp.tile([128, DIAGW], F32, tag="bmask")
            warm = constp.tile([128, 1], F32, tag="warm")

            # one-time init (gpsimd): zero the d-pad rows of Q/K tiles,
            # set the ones column of V tiles (softmax denominator), and
            # build the fused exp+mask bias tile.
            for t in qts + kts:
                nc.gpsimd.memset(t[64:128, :], 0.0)
            for t in vps:
                nc.gpsimd.memset(t[:, :, D : D + 1], 1.0)
            nc.gpsimd.memset(bmask, float(B16))
            # keep (B16) where partition p <= local col j, else MASKB
            nc.gpsimd.affine_select(
                out=bmask[:, 0:128],
                in_=bmask[:, 0:128],
                compare_op=mybir.AluOpType.is_ge,
                fill=float(MASKB),
                base=0,
                pattern=[[1, 128]],
                channel_multiplier=-1,
            )
            # warm the scalar engine's Exp table before the pipeline
            nc.gpsimd.memset(warm, 0.0)
            nc.scalar.activation(
                out=warm, in_=warm, func=mybir.ActivationFunctionType.Exp
            )

            # ---- DMA issue (HWDGE on sync; no casts needed) ----------
            def issue_qk(h, split=1):
                qt, kt = qts[h % 2], kts[h % 2]
                step = S // split
                for c in range(0, S, step):
                    nc.sync.dma_start(
                        out=kt[0:64, c : c + step], in_=kt_d[h][:, c : c + step]
                    )
                    nc.sync.dma_start(
                        out=qt[0:64, c : c + step], in_=qt_d[h][:, c : c + step]
                    )

            def issue_v(h):
                nc.sync.dma_start(out=vps[h % N_VBUF][:, :, 0:D], in_=v_r[h])

            issue_qk(0, split=4)
            if n_heads > 1:
                issue_qk(1)
            for h in range(min(3, n_heads)):
                issue_v(h)

            class PvEmitter:
                """PV matmuls for one head in (qb, kb2) order. O and the
                softmax denominator accumulate together in PSUM ([...,64]
                is the ones-column product). Two q-blocks share one PSUM
                tile; normalize = DVE reciprocal + gpsimd multiply."""

                def __init__(self, uts, vp, oh):
                    self.uts, self.vp, self.oh = uts, vp, oh
                    self.pairs = [
                        (qb, kb2) for qb in range(NB) for kb2 in range(qb + 1)
                    ]
                    self.pos = 0
                    self.po2 = None

                def emit_to(self, n):
                    for qb, kb2 in self.pairs[self.pos : n]:
                        if kb2 == 0 and qb % 2 == 0:
                            self.po2 = ps_o.tile([128, 2, D + 2], F32, tag="o")
                        po = self.po2[:, qb % 2, :]
                        nc.tensor.matmul(
                            po[:, 0 : D + 1],
                            lhsT=self.uts[kb2][
                                :, (qb - kb2) * 128 : (qb - kb2) * 128 + 128
                            ],
                            rhs=self.vp[:, kb2, 0 : D + 1],
                            start=(kb2 == 0),
                            stop=(kb2 == qb),
                        )
                        if kb2 == qb and (qb % 2 == 1 or qb == NB - 1):
                            # close the PSUM pair: one reciprocal over both
                            # denominator columns (DVE), then per-qb
                            # normalize multiplies on the scalar engine
                            # (gpsimd can't read PSUM).
                            n_in_pair = qb % 2 + 1
                            rz2 = rzp.tile([128, 2, 1], F32, tag="rz")
                            nc.vector.reciprocal(
                                rz2[:, 0:n_in_pair, :],
                                self.po2[:, 0:n_in_pair, D : D + 1],
                            )
                            for i in range(n_in_pair):
                                nc.scalar.activation(
                                    out=self.oh[:, qb - n_in_pair + 1 + i, :],
                                    in_=self.po2[:, i, 0:D],
                                    func=mybir.ActivationFunctionType.Copy,
                                    scale=rz2[:, i, :],
                                )
                    self.pos = max(self.pos, min(n, len(self.pairs)))

                def remaining(self):
                    return len(self.pairs) - self.pos

            N_SLOTS = sum(-(-(S - kb * 128) // 1024) for kb in range(NB))  # 24
            N_PAIRS = NB * (NB + 1) // 2  # 136

            prev = None  # (PvEmitter, oh) of head h-1
            for h in range(n_heads + 1):
                cur = None
                if h < n_heads:
                    if h + 2 < n_heads:
                        issue_qk(h + 2)
                    if h + 3 < n_heads:
                        issue_v(h + 3)
                    qt, kt = qts[h % 2], kts[h % 2]
                    vp = vps[h % N_VBUF]
                    oh = ohs[h % 2]
                    uts = []
                    cur = (PvEmitter(uts, vp, oh), oh)

                slot = 0
                for kb in range(NB if h < n_heads else 0):
                    qlo = kb * 128
                    L = S - qlo
                    ut = utp.tile([128, L], BF16, tag=f"ut{kb}")
                    uts.append(ut)
                    for t0 in range(0, L, 1024):
                        tl = min(1024, L - t0)
                        ps = ps_s.tile([128, 1024], F32, tag="s")
                        for cc in range(0, tl, 512):
                            cl = min(512, tl - cc)
                            nc.tensor.matmul(
                                ps[:, cc : cc + cl],
                                lhsT=kt[:, qlo : qlo + 128],
                                rhs=qt[
                                    :, qlo + t0 + cc : qlo + t0 + cc + cl
                                ],
                                start=True,
                                stop=True,
                            )
                        for c0, w, eng in SLOT_PLAN[kb]:
                            if not (t0 <= c0 < t0 + tl):
                                continue
                            rel = c0 - t0
                            if eng == "diag":
                                # fused exp + causal mask of the diagonal
                                # 128-block: (ps*A16) + BMASK -> i16 bits
                                # of bf16 exp
                                nc.vector.scalar_tensor_tensor(
                                    out=ut[:, c0 : c0 + w].bitcast(I16),
                                    in0=ps[:, rel : rel + w],
                                    scalar=float(A16),
                                    in1=bmask[:, 0:w],
                                    op0=mybir.AluOpType.mult,
                                    op1=mybir.AluOpType.add,
                                )
                            elif eng == "V":
                                nc.vector.tensor_scalar(
                                    out=ut[:, c0 : c0 + w].bitcast(I16),
                                    in0=ps[:, rel : rel + w],
                                    scalar1=float(A16),
                                    scalar2=float(B16),
                                    op0=mybir.AluOpType.mult,
                                    op1=mybir.AluOpType.add,
                                )
                            else:
                                nc.scalar.activation(
                                    out=ut[:, c0 : c0 + w],
                                    in_=ps[:, rel : rel + w],
                                    func=mybir.ActivationFunctionType.Exp,
                                    scale=float(SCALE),
                                )
                        slot += 1
                        if prev is not None:
                            prev[0].emit_to((N_PAIRS * slot) // N_SLOTS)

                if prev is not None:
                    pv, ohprev = prev
                    pv.emit_to(N_PAIRS)
                    nc.sync.dma_start(out=o_r[h - 1], in_=ohprev)
                prev = cur
    _split_multi_waits(nc)
    return nc


_NC_CACHE = {}


def _get_nc(n_heads: int = HEADS_PER_CORE):
    if n_heads not in _NC_CACHE:
        _NC_CACHE[n_heads] = build_nc(n_heads)
    return _NC_CACHE[n_heads]


def make_in_maps(queries, keys, values):
    # host-side input marshaling: flatten (B,H), cast to bf16, and
    # pre-transpose Q, K to [64, S] so the device needs no transposes
    # and no casting DMAs.
    import ml_dtypes

    bf16 = ml_dtypes.bfloat16
    qf = np.asarray(queries, dtype=np.float32).reshape(B * H, S, D)
    kf = np.asarray(keys, dtype=np.float32).reshape(B * H, S, D)
    qt = np.ascontiguousarray(qf.transpose(0, 2, 1)).astype(bf16)
    kt = np.ascontiguousarray(kf.transpose(0, 2, 1)).astype(bf16)
    vf = np.ascontiguousarray(
        np.asarray(values, dtype=np.float32).reshape(B * H, S, D)
    ).astype(bf16)
    n = HEADS_PER_CORE
    return [
        {
            "queriesT": qt[i * n : (i + 1) * n],
            "keysT": kt[i * n : (i + 1) * n],
            "values": vf[i * n : (i + 1) * n],
        }
        for i in range(N_CORES)
    ]


def kernel(keys, queries, values, head_dim=None, **_ignored):
    nc = _get_nc()
    in_maps = make_in_maps(queries, keys, values)
    res = run_bass_kernel_spmd(nc, in_maps, core_ids=list(range(N_CORES)))
    out = np.concatenate([res.results[i]["out"] for i in range(N_CORES)], axis=0)
    return out.reshape(B, H, S, D).astype(np.float32)


# revision 6
# speedup vs baseline: 1.2091x; 1.2091x over previous
"""Causal multi-head attention (B=4, H=16, S=2048, D=64) on 8 TRN2 NeuronCores.

Sharding: B*H = 64 (batch, head) pairs -> 8 per core, fully independent,
no collectives.

v3 design (evolved from the 174us v1 via trace analysis):
  - Host pre-casts Q,K,V to bf16; Q,K pre-transposed to [128, S] (d on
    partitions, rows 64:128 zero). Input DMA ~12MB/core (vs 36MB in v1)
    and cast-free.
  - Input DMAs issue on gpsimd (SWDGE) so their write-after-read waits
    block only the Pool queue; output DMAs issue on sync (HWDGE).
    Q/K/V/O SBUF tiles come from per-head tile() ring tags so the tile
    framework inserts the WAR hazards on slot reuse.
  - exp is split between the Scalar engine (exact, activation Exp) and
    the DVE (one-pass i16 Schraudolph: bits16 = round(A16*s + B16)
    written via f32->i16 convert into the bf16 ut tile). ~2% rms on the
    DVE share, same as v1's two-pass i32 path, at half the DVE cost.
  - The causal diagonal-block mask is FOLDED INTO the DVE exp: the
    first 256 cols of each key-block row use scalar_tensor_tensor
    (ps*A16) + BMASK, where BMASK holds B16 on the kept triangle and
    B16 + A16*(-600) on the masked part (masked probs ~1e-33). v1's
    trimask multiply (38us DVE) disappears.
  - Normalize: one reciprocal per PSUM pair-tile (two q-blocks share a
    PSUM tile, 4 q-blocks in flight) + per-q-block multiply, all DVE.
  - Key blocks are processed in REVERSE (kb 15..0): head 0's Q/K DMAs
    land tail-chunk-first so the first (short) score rows start after
    ~2us instead of waiting for the full [128,2048] transfer.
  - PV for head h-1 is interleaved after every score tile (~6 pairs per
    slot) to keep the PE stream dense (p-state!).
"""

import numpy as np

import concourse.bass as bass
import concourse.tile as tile
from concourse import mybir
from concourse.bass_utils import run_bass_kernel_spmd
from concourse.vector_clock import ScopedClock, VectorClock

F32 = mybir.dt.float32
BF16 = mybir.dt.bfloat16
I16 = mybir.dt.int16

B, H, S, D = 4, 16, 2048, 64
N_CORES = 8
HEADS_PER_CORE = B * H // N_CORES  # 8
NB = S // 128  # 16 key blocks of 128
SCALE = 1.0 / np.sqrt(np.float32(D))  # 0.125
DIAGW = 256  # width of the fused-mask DVE slot at the head of each kb row

# i16 Schraudolph: bits16 = round(A16*s + B16) viewed as bf16 ~ exp(s/8)
A16 = 0.125 * float(np.log2(np.e)) * 128.0  # 23.0831
B16 = (127.0 - 0.0440) * 128.0  # 16250.368
MASK_BIAS = -600.0  # exp(0.125*(s-600)) ~ 1e-33: dead but positive bf16
MASKB = B16 + A16 * MASK_BIAS  # ~2400.5: tiny positive bf16 bits

# measured per-slot engine costs (ns) for the static scalar/DVE balance
_SC_NS = lambda w: 0.834 * w + 95.0
_DV_NS = lambda w: 1.07 * w + 170.0


def _plan_slots():
    """Per kb: list of (c0, w, engine) exp slots; engine in {'diag','S','V'}.
    Greedy-balance the flexible slots across Scalar and DVE given each
    engine's fixed load (DVE: diag slots + paired reciprocals +
    normalize multiplies)."""
    slots = {}
    flex = []
    dve_t = 8 * 137.0 + 16 * 140.0  # reciprocals + normalize multiplies
    sc_t = 0.0
    for kb in range(NB):
        L = S - kb * 128
        dw = min(DIAGW, L)
        slots[kb] = [(0, dw, "diag")]
        dve_t += _DV_NS(dw)
        c = dw
        while c < L:
            # flex chunks end at ps-tile boundaries (multiples of 1024)
            w = min(1024 * (c // 1024 + 1), L) - c
            flex.append((kb, c, w))
            c += w
    for kb, c, w in sorted(flex, key=lambda t: -t[2]):
        if sc_t + _SC_NS(w) <= dve_t + _DV_NS(w):
            slots[kb].append((c, w, "S"))
            sc_t += _SC_NS(w)
        else:
            slots[kb].append((c, w, "V"))
            dve_t += _DV_NS(w)
    for kb in slots:
        slots[kb].sort()
    return slots


SLOT_PLAN = _plan_slots()


def _patch_tile_drain():
    """This walrus build rejects >1 sem wait on the kernel-tail Drain
    instruction ("Too many sync wait commands"). Spread the waits across
    single-wait NOPs on the sync engine instead."""
    if getattr(tile.TileContext, "_drain_patched", False):
        return

    def _drain_and_barrier(self, tick_clock, wait_clock):
        gc = tick_clock.global_clock
        n = len(gc)
        for i in range(n):
            if gc[i] > 0:
                vc = VectorClock([gc[j] if j == i else 0 for j in range(n)])
                nop_inst = self.nc.sync.nop(nofuse=True, hint=f"drainwait{i}")
                wait_clock.add_sem_waits(nop_inst.ins, ScopedClock({None: vc}))
        self.nc.sync.drain()
        self.nc.all_engine_barrier()
        popped = self.nc._tile_sem_poison_stack.pop()
        assert popped is self._sem_poison
        self.nc.clear_and_free_semaphores(list(self.sems.allocated().values()))
        self.nc.all_engine_barrier()

    tile.TileContext._drain_and_barrier = _drain_and_barrier
    tile.TileContext._drain_patched = True


_patch_tile_drain()


def _split_multi_waits(nc, limit=1):
    """This walrus build allows at most one sem wait per instruction.
    Move excess waits onto same-engine NOPs inserted just before."""
    ctr = [0]
    for func in nc.m.functions:
        for bb in func.blocks:
            insts = list(bb.instructions)
            out = []
            changed = False
            for inst in insts:
                si = inst.sync_info
                if si is not None and si.on_wait is not None and len(si.on_wait) > limit:
                    waits = list(si.on_wait)
                    extra, keep = waits[:-limit], waits[-limit:]
                    for w in extra:
                        ctr[0] += 1
                        nop = mybir.InstNoOp(
                            name=f"waitsplit-{ctr[0]}", ins=[], outs=[]
                        )
                        nop.engine = inst.engine
                        nop.sync_info = mybir.SyncInfo(on_wait=[w], on_update=[])
                        out.append(nop)
                    inst.sync_info = mybir.SyncInfo(
                        on_wait=keep, on_update=list(si.on_update or [])
                    )
                    changed = True
                out.append(inst)
            if changed:
                try:
                    bb.instructions[:] = out
                except Exception:
                    bb.instructions = out
    return nc


def build_nc(n_heads: int = HEADS_PER_CORE):
    nc = bass.Bass("TRN2", target_bir_lowering=False)
    qt_d = nc.dram_tensor("queriesT", [n_heads, 128, S], BF16, kind="ExternalInput")
    kt_d = nc.dram_tensor("keysT", [n_heads, 128, S], BF16, kind="ExternalInput")
    v_d = nc.dram_tensor("values", [n_heads, S, D], BF16, kind="ExternalInput")
    o_d = nc.dram_tensor("out", [n_heads, S, D], F32, kind="ExternalOutput")

    # [h, p, n, d] view of v / out: s = n*128 + p
    v_r = v_d[:].rearrange("h (n p) d -> h p n d", p=128)
    o_r = o_d[:].rearrange("h (n p) d -> h p n d", p=128)

    KB_ORDER = list(range(NB - 1, -1, -1))  # 15..0: tail rows first

    with tile.TileContext(nc) as tc:
        with (
            tc.tile_pool(name="const", bufs=1) as constp,
            tc.tile_pool(name="tp", bufs=2) as tpp,
            tc.tile_pool(name="vpool", bufs=4) as vpp,
            tc.tile_pool(name="ut", bufs=2) as utp,
            tc.tile_pool(name="oh", bufs=2) as ohp,
            tc.tile_pool(name="rz", bufs=4) as rzp,
            tc.tile_pool(name="ps_s", bufs=3, space="PSUM") as ps_s,
            tc.tile_pool(name="ps_o", bufs=2, space="PSUM") as ps_o,
        ):
            bmask = constp.tile([128, DIAGW], F32, tag="bmask")
            warm = constp.tile([128, 1], F32, tag="warm")

            # one-time init: fused exp+mask bias tile; warm the scalar
            # engine's Exp table.
            nc.gpsimd.memset(bmask, float(B16))
            # keep (B16) where partition p <= local col j, else MASKB
            nc.gpsimd.affine_select(
                out=bmask[:, 0:128],
                in_=bmask[:, 0:128],
                compare_op=mybir.AluOpType.is_ge,
                fill=float(MASKB),
                base=0,
                pattern=[[1, 128]],
                channel_multiplier=-1,
            )
            nc.gpsimd.memset(warm, 0.0)
            nc.scalar.activation(
                out=warm, in_=warm, func=mybir.ActivationFunctionType.Exp
            )

            xps = {}
            vps = {}

            # ---- DMA issue (SWDGE on gpsimd: parallel to sync queue) --
            def issue_qk(h, split=1):
                qt = tpp.tile([128, S], BF16, tag=f"qt{h % 2}")
                kt = tpp.tile([128, S], BF16, tag=f"kt{h % 2}")
                step = S // split
                # reversed chunk order: tail columns land first, matching
                # the kb 15..0 processing order
                for c in range(S - step, -1, -step):
                    nc.gpsimd.dma_start(
                        out=kt[:, c : c + step], in_=kt_d[h][:, c : c + step]
                    )
                    nc.gpsimd.dma_start(
                        out=qt[:, c : c + step], in_=qt_d[h][:, c : c + step]
                    )
                xps[h] = (qt, kt)

            def issue_v(h):
                vp = vpp.tile([128, NB, D + 2], BF16, tag="vp")
                nc.gpsimd.dma_start(out=vp[:, :, 0:D], in_=v_r[h])
                nc.gpsimd.memset(vp[:, :, D : D + 1], 1.0)
                vps[h] = vp

            issue_qk(0, split=4)
            if n_heads > 1:
                issue_qk(1)
            for h in range(min(3, n_heads)):
                issue_v(h)

            class PvEmitter:
                """PV matmuls for one head in (qb, kb2) order. O and the
                softmax denominator accumulate together in PSUM (col 64
                is the ones-column product). Two q-blocks share one PSUM
                tile; normalize = paired DVE reciprocal + DVE multiply."""

                def __init__(self, uts, vp, oh):
                    self.uts, self.vp, self.oh = uts, vp, oh
                    self.pairs = [
                        (qb, kb2) for qb in range(NB) for kb2 in range(qb + 1)
                    ]
                    self.pos = 0
                    self.po2 = None

                def emit_to(self, n):
                    for qb, kb2 in self.pairs[self.pos : n]:
                        if kb2 == 0 and qb % 2 == 0:
                            self.po2 = ps_o.tile([128, 2, D + 2], F32, tag="o")
                        po = self.po2[:, qb % 2, :]
                        nc.tensor.matmul(
                            po[:, 0 : D + 1],
                            lhsT=self.uts[kb2][
                                :, (qb - kb2) * 128 : (qb - kb2) * 128 + 128
                            ],
                            rhs=self.vp[:, kb2, 0 : D + 1],
                            start=(kb2 == 0),
                            stop=(kb2 == qb),
                        )
                        if kb2 == qb and qb % 2 == 1:
                            # close the PSUM pair: one reciprocal over
                            # both denominator columns, then per-qb
                            # normalize multiplies (all DVE; gpsimd
                            # can't read PSUM).
                            rz2 = rzp.tile([128, 2, 1], F32, tag="rz")
                            nc.vector.reciprocal(
                                rz2, self.po2[:, :, D : D + 1]
                            )
                            for i in (0, 1):
                                nc.vector.tensor_scalar_mul(
                                    self.oh[:, qb - 1 + i, :],
                                    self.po2[:, i, 0:D],
                                    rz2[:, i, :],
                                )
                    self.pos = max(self.pos, min(n, len(self.pairs)))

            N_SLOTS = sum(-(-(S - kb * 128) // 1024) for kb in range(NB))  # 24
            N_PAIRS = NB * (NB + 1) // 2  # 136

            prev = None  # PvEmitter of head h-1
            for h in range(n_heads + 1):
                cur = None
                if h < n_heads:
                    if h + 2 < n_heads:
                        issue_qk(h + 2)
                    if h + 3 < n_heads:
                        issue_v(h + 3)
                    qt, kt = xps.pop(h)
                    vp = vps.pop(h)
                    oh = ohp.tile([128, NB, D], F32, tag="oh")
                    uts = {}
                    cur = (PvEmitter(uts, vp, oh), oh)

                slot = 0
                for kb in (KB_ORDER if h < n_heads else []):
                    qlo = kb * 128
                    L = S - qlo
                    ut = utp.tile([128, L], BF16, tag=f"ut{kb}")
                    uts[kb] = ut
                    for t0 in range(0, L, 1024):
                        tl = min(1024, L - t0)
                        ps = ps_s.tile([128, 1024], F32, tag="s")
                        for cc in range(0, tl, 512):
                            cl = min(512, tl - cc)
                            nc.tensor.matmul(
                                ps[:, cc : cc + cl],
                                lhsT=kt[:, qlo : qlo + 128],
                                rhs=qt[
                                    :, qlo + t0 + cc : qlo + t0 + cc + cl
                                ],
                                start=True,
                                stop=True,
                            )
                        for c0, w, eng in SLOT_PLAN[kb]:
                            if not (t0 <= c0 < t0 + tl):
                                continue
                            rel = c0 - t0
                            if eng == "diag":
                                # fused exp + causal mask of the diagonal
                                # 128-block: (ps*A16) + BMASK -> i16 bits
                                # of bf16 exp
                                nc.vector.scalar_tensor_tensor(
                                    out=ut[:, c0 : c0 + w].bitcast(I16),
                                    in0=ps[:, rel : rel + w],
                                    scalar=float(A16),
                                    in1=bmask[:, 0:w],
                                    op0=mybir.AluOpType.mult,
                                    op1=mybir.AluOpType.add,
                                )
                            elif eng == "V":
                                nc.vector.tensor_scalar(
                                    out=ut[:, c0 : c0 + w].bitcast(I16),
                                    in0=ps[:, rel : rel + w],
                                    scalar1=float(A16),
                                    scalar2=float(B16),
                                    op0=mybir.AluOpType.mult,
                                    op1=mybir.AluOpType.add,
                                )
                            else:
                                nc.scalar.activation(
                                    out=ut[:, c0 : c0 + w],
                                    in_=ps[:, rel : rel + w],
                                    func=mybir.ActivationFunctionType.Exp,
                                    scale=float(SCALE),
                                )
                        slot += 1
                        if prev is not None:
                            prev[0].emit_to((N_PAIRS * slot) // N_SLOTS)

                if prev is not None:
                    pv, ohprev = prev
                    pv.emit_to(N_PAIRS)
                    nc.sync.dma_start(out=o_r[h - 1], in_=ohprev)
                prev = cur
    _split_multi_waits(nc)
    return nc


_NC_CACHE = {}


def _get_nc(n_heads: int = HEADS_PER_CORE):
    if n_heads not in _NC_CACHE:
        _NC_CACHE[n_heads] = build_nc(n_heads)
    return _NC_CACHE[n_heads]


def make_in_maps(queries, keys, values):
    # host-side input marshaling: flatten (B,H), cast to bf16, and
    # pre-transpose Q, K to [128, S] (rows 64:128 zero) so the device
    # needs no transposes, no casting DMAs, and no pad memsets.
    import ml_dtypes

    bf16 = ml_dtypes.bfloat16
    qf = np.asarray(queries, dtype=np.float32).reshape(B * H, S, D)
    kf = np.asarray(keys, dtype=np.float32).reshape(B * H, S, D)
    qt = np.zeros((B * H, 128, S), dtype=bf16)
    kt = np.zeros((B * H, 128, S), dtype=bf16)
    qt[:, 0:D, :] = qf.transpose(0, 2, 1).astype(bf16)
    kt[:, 0:D, :] = kf.transpose(0, 2, 1).astype(bf16)
    vf = np.ascontiguousarray(
        np.asarray(values, dtype=np.float32).reshape(B * H, S, D)
    ).astype(bf16)
    n = HEADS_PER_CORE
    return [
        {
            "queriesT": qt[i * n : (i + 1) * n],
            "keysT": kt[i * n : (i + 1) * n],
            "values": vf[i * n : (i + 1) * n],
        }
        for i in range(N_CORES)
    ]


def kernel(keys, queries, values, head_dim=None, **_ignored):
    nc = _get_nc()
    in_maps = make_in_maps(queries, keys, values)
    res = run_bass_kernel_spmd(nc, in_maps, core_ids=list(range(N_CORES)))
    out = np.concatenate([res.results[i]["out"] for i in range(N_CORES)], axis=0)
    return out.reshape(B, H, S, D).astype(np.float32)
